# revision 1
# baseline (speedup 1.0000x reference)
"""KAN (B-spline) network kernel for 8 Trainium2 NeuronCores.

Strategy:
- Data-parallel over batch: 8192 rows -> 1024 per core; weights replicated
  (embedded in the NEFF as Const tensors).
- Activations kept transposed on-chip: (feature, batch) with batch tiles of
  512 in the free dimension.
- Spline term evaluated via truncated-power form: for u = (x-lo)/h + 3
  clamped to <= 16,  sum_g N3(u-g)*D[g] == sum_{s=0..16} beta_s * relu(u-s)^3.
  relu pass on DVE (fused sub+max tensor_scalar), square on ACT, cube on
  DVE/GPSIMD, then fp32 matmuls against host-precomputed beta matrices.
- Base term: mish(x) = x * tanh(softplus(x)) computed exactly via the
  identity tanh(softplus(x)) = 1 - 2/((e^x+1)^2+1) using Exp/Square/Ln
  activations (single ACT table set, inf-safe for large x).
- log_softmax on device (PE transpose + Exp/Ln + DVE reductions).
"""
import sys
import os

sys.path.insert(0, '/opt/trn_rl_repo')

import numpy as np
from contextlib import ExitStack

import concourse.bass as bass
import concourse.bacc as bacc
import concourse.tile as tile
from concourse import mybir
from concourse.bass_utils import run_bass_kernel_spmd

F32 = mybir.dt.float32
AF = mybir.ActivationFunctionType
ALU = mybir.AluOpType

N_CORES = 8
B_TOTAL = 8192
B_CORE = B_TOTAL // N_CORES     # 1024
BT = 512                        # batch tile (free dim)
NBT = B_CORE // BT              # 2
K_ORD, GRID = 3, 10
LO, HI = -2.0, 2.0
H = (HI - LO) / GRID            # 0.4
NC_B = GRID + K_ORD             # 13 basis functions
NS = 17                         # truncated-power slots s = 0..16
USC, UOF = 1.0 / H, K_ORD - LO / H   # u = x/H + (3 - LO/H) = 2.5x + 8

_CACHE = {}


def _beta(coef, sp):
    """R-form coefficients: beta[i, s, o] with
    sum_g D[i,g,o] N3(u-g) = sum_s beta[i,s,o] relu(u-s)^3 for u in [0,16]."""
    D = (coef * sp[..., None]).astype(np.float64)          # (in, out, 13)
    c = np.array([1.0, -4.0, 6.0, -4.0, 1.0]) / 6.0
    fin, fout = D.shape[0], D.shape[1]
    beta = np.zeros((fin, NS, fout))
    for g in range(NC_B):
        for r in range(5):
            beta[:, g + r, :] += c[r] * D[:, :, g]
    return beta.astype(np.float32)


def _build(weights):
    nc = bacc.Bacc("TRN2", target_bir_lowering=False, debug=False,
                   num_devices=N_CORES)
    xT = nc.dram_tensor("xT", [49, B_CORE], F32, kind="ExternalInput")
    out_d = nc.dram_tensor("out", [B_CORE, 10], F32, kind="ExternalOutput")
    dbg = {}
    if os.environ.get("KDBG"):
        for n, shp in [("uc1", [98, BT]), ("cu1", [98, 9 * BT]),
                       ("mish1", [49, BT]), ("h2_0", [128, BT]),
                       ("h2_1", [128, BT]), ("h3_0", [128, BT]),
                       ("cu2_0", [128, NS * BT]), ("mish2_0", [128, BT])]:
            dbg[n] = nc.dram_tensor("dbg_" + n, shp, F32, kind="ExternalOutput")

    # ---- host-precomputed constants -> NEFF Const tensors ----
    b1 = weights['b1']; b2 = weights['b2']; b3 = weights['b3']
    beta1 = _beta(weights['coef1'], weights['sp1'])    # (49, 17, 256)
    beta2 = _beta(weights['coef2'], weights['sp2'])    # (256, 17, 256)
    beta3 = _beta(weights['coef3'], weights['sp3'])    # (256, 17, 10)

    # L1 two-pack: rows p<49 -> (i=p, s=2j), p>=49 -> (i=p-49, s=2j+1)
    NJ1 = 9
    e1 = np.zeros((98, NJ1, 256), np.float32)
    for j in range(NJ1):
        e1[:49, j, :] = beta1[:, 2 * j, :]
        if 2 * j + 1 < NS:
            e1[49:, j, :] = beta1[:, 2 * j + 1, :]
    # negS for L1 relu ops: s value per partition for each j
    s1v = np.zeros((98, NJ1), np.float32)
    for j in range(NJ1):
        s1v[:49, j] = 2 * j
        s1v[49:, j] = 2 * j + 1

    consts = {
        'e1': e1.reshape(98, NJ1 * 256),
        's1v': s1v,
        'e2': np.ascontiguousarray(beta2.reshape(2, 128, NS * 256)),
        'e3': np.ascontiguousarray(beta3.reshape(2, 128, NS * 10)),
        'sb1': weights['sb1'].astype(np.float32),               # (49,256)
        'sb2': weights['sb2'].astype(np.float32),               # (256,256)
        'sb3': weights['sb3'].astype(np.float32),               # (256,10)
        'bias1': b1.reshape(2, 128, 1).astype(np.float32),
        'bias2': b2.reshape(2, 128, 1).astype(np.float32),
        'bias3': b3.reshape(10, 1).astype(np.float32),
        'ubias1': (USC * b1 + UOF).reshape(2, 128, 1).astype(np.float32),
        'ubias2': (USC * b2 + UOF).reshape(2, 128, 1).astype(np.float32),
        'eye': np.eye(128, dtype=np.float32),
    }
    dts = {k: nc.inline_tensor(v, name=k) for k, v in consts.items()}

    with tile.TileContext(nc) as tc, ExitStack() as ctx:
        wpool = ctx.enter_context(tc.tile_pool(name="w", bufs=1))
        # resident weight tiles
        e1t = wpool.tile([98, NJ1 * 256], F32)
        nc.sync.dma_start(e1t[:], dts['e1'].ap())
        s1t = wpool.tile([98, NJ1], F32)
        nc.sync.dma_start(s1t[:], dts['s1v'].ap())
        e2t = [wpool.tile([128, NS * 256], F32, tag=f"e2_{ic}", name=f"e2_{ic}") for ic in range(2)]
        for ic in range(2):
            nc.sync.dma_start(e2t[ic][:], dts['e2'].ap()[ic])
        e3t = [wpool.tile([128, NS * 10], F32, tag=f"e3_{ic}", name=f"e3_{ic}") for ic in range(2)]
        for ic in range(2):
            nc.sync.dma_start(e3t[ic][:], dts['e3'].ap()[ic])
        sb1t = wpool.tile([49, 256], F32)
        nc.sync.dma_start(sb1t[:], dts['sb1'].ap())
        sb2t = [wpool.tile([128, 256], F32, tag=f"sb2_{ic}", name=f"sb2_{ic}") for ic in range(2)]
        for ic in range(2):
            nc.sync.dma_start(sb2t[ic][:], dts['sb2'].ap()[ic * 128:(ic + 1) * 128, :])
        sb3t = [wpool.tile([128, 10], F32, tag=f"sb3_{ic}", name=f"sb3_{ic}") for ic in range(2)]
        for ic in range(2):
            nc.sync.dma_start(sb3t[ic][:], dts['sb3'].ap()[ic * 128:(ic + 1) * 128, :])
        bias2t = [wpool.tile([128, 1], F32, tag=f"b2_{oc}", name=f"b2_{oc}") for oc in range(2)]
        ubias2t = [wpool.tile([128, 1], F32, tag=f"ub2_{oc}", name=f"ub2_{oc}") for oc in range(2)]
        for oc in range(2):
            nc.sync.dma_start(bias2t[oc][:], dts['bias2'].ap()[oc])
            nc.sync.dma_start(ubias2t[oc][:], dts['ubias2'].ap()[oc])
        bias1t = [wpool.tile([128, 1], F32, tag=f"b1_{oc}", name=f"b1_{oc}") for oc in range(2)]
        ubias1t = [wpool.tile([128, 1], F32, tag=f"ub1_{oc}", name=f"ub1_{oc}") for oc in range(2)]
        for oc in range(2):
            nc.sync.dma_start(bias1t[oc][:], dts['bias1'].ap()[oc])
            nc.sync.dma_start(ubias1t[oc][:], dts['ubias1'].ap()[oc])
        bias3t = wpool.tile([10, 1], F32)
        nc.sync.dma_start(bias3t[:], dts['bias3'].ap())
        eyet = wpool.tile([128, 128], F32)
        nc.sync.dma_start(eyet[:], dts['eye'].ap())

        io = ctx.enter_context(tc.tile_pool(name="io", bufs=2))
        wide = ctx.enter_context(tc.tile_pool(name="wide", bufs=1))
        nar = ctx.enter_context(tc.tile_pool(name="nar", bufs=1))
        ps = ctx.enter_context(tc.tile_pool(name="ps", bufs=1, space="PSUM"))
        sm = ctx.enter_context(tc.tile_pool(name="sm", bufs=2))

        def mish_of(h_src, bias_ap, parts, blk):
            """mish tile (parts,BT) from psum/sbuf h_src (+bias).
            tanh(softplus(h)) = 1 - 2/((e^h+1)^2+1); h clamped at 40 before
            Exp: Ln table domain is +-2^64 so (e^h+1)^2 must stay below it;
            the correction term underflows to 0 beyond h=21 anyway."""
            h = nar.tile([parts, BT], F32, tag="h", name=f"h{blk}")
            if bias_ap is None:
                nc.vector.tensor_copy(h[:], h_src)
            else:
                nc.vector.tensor_scalar(h[:], h_src, bias_ap, None, ALU.add)
            hc = nar.tile([parts, BT], F32, tag="hc", name=f"hc{blk}")
            nc.vector.tensor_scalar(hc[:], h[:], 21.0, None, ALU.min)
            z = nar.tile([parts, BT], F32, tag="z", name=f"z{blk}")
            nc.scalar.activation(z[:], hc[:], AF.Exp)
            s2 = nar.tile([parts, BT], F32, tag="s2", name=f"s2{blk}")
            nc.scalar.activation(s2[:], z[:], AF.Square, bias=1.0)
            ll = nar.tile([parts, BT], F32, tag="ll", name=f"ll{blk}")
            nc.scalar.activation(ll[:], s2[:], AF.Ln, bias=1.0)
            rr = nar.tile([parts, BT], F32, tag="rr", name=f"rr{blk}")
            nc.scalar.activation(rr[:], ll[:], AF.Exp, scale=-1.0)
            w = nar.tile([parts, BT], F32, tag="w", name=f"w{blk}")
            nc.vector.tensor_scalar(w[:], rr[:], -2.0, 1.0, ALU.mult, ALU.add)
            m = nar.tile([parts, BT], F32, tag=f"m{blk}", name=f"m{blk}")
            nc.vector.tensor_mul(m[:], h[:], w[:])
            mish_of.last_h = h
            return m

        def wide_powers(uc, parts, nslot, s_imm, s_ap, blk, cube_on_pool):
            """r=relu(uc-s), sq=r^2, r<-sq*r in place; returns cube tile."""
            r = wide.tile([parts, nslot * BT], F32, tag="r", name=f"r{blk}",
                          bufs=2)
            for j in range(nslot):
                sl = r[:, j * BT:(j + 1) * BT]
                if s_ap is not None:
                    nc.vector.tensor_scalar(sl, uc[:], s_ap[:, j:j + 1], 0.0,
                                            ALU.subtract, ALU.max)
                else:
                    nc.vector.tensor_scalar(sl, uc[:], float(s_imm[j]), 0.0,
                                            ALU.subtract, ALU.max)
            sq = wide.tile([parts, nslot * BT], F32, tag="sq", name=f"sq{blk}",
                           bufs=1)
            nc.scalar.activation(sq[:], r[:], AF.Square)
            if cube_on_pool:
                nc.gpsimd.tensor_mul(r[:], sq[:], r[:])
            else:
                nc.vector.tensor_mul(r[:], sq[:], r[:])
            return r

        for bt in range(NBT):
            bsl = slice(bt * BT, (bt + 1) * BT)
            # ---- load x tile (49 rows, duplicated into 98 partitions) ----
            xt = io.tile([98, BT], F32, tag="xt", name="xt")
            nc.sync.dma_start(xt[0:49, :], xT.ap()[:, bsl])
            nc.sync.dma_start(xt[49:98, :], xT.ap()[:, bsl])
            # u1 = clamp(2.5x + 8, None, 16)
            ua = nar.tile([98, BT], F32, tag="ua", name="ua1")
            nc.vector.tensor_scalar(ua[:], xt[:], USC, UOF, ALU.mult, ALU.add)
            uc1 = nar.tile([98, BT], F32, tag="uc1", name="uc1")
            nc.vector.tensor_scalar(uc1[:], ua[:], 16.0, None, ALU.min)

            cu1 = wide_powers(uc1, 98, NJ1, None, s1t, "L1", cube_on_pool=False)
            mish1 = mish_of(xt[0:49, :], None, 49, "L1")
            if dbg and bt == 0:
                nc.sync.dma_start(dbg["uc1"][:], uc1[:])
                nc.sync.dma_start(dbg["cu1"][:], cu1[:])
                nc.sync.dma_start(dbg["mish1"][:], mish1[:])

            ps1 = [ps.tile([128, BT], F32, tag=f"ps1_{oc}", name=f"ps1_{oc}") for oc in range(2)]
            for oc in range(2):
                for j in range(NJ1):
                    nc.tensor.matmul(
                        ps1[oc][:],
                        e1t[:, j * 256 + oc * 128: j * 256 + (oc + 1) * 128],
                        cu1[:, j * BT:(j + 1) * BT],
                        start=(j == 0), stop=False)
                nc.tensor.matmul(ps1[oc][:], sb1t[:, oc * 128:(oc + 1) * 128],
                                 mish1[:], start=False, stop=True)

            # ---- layer 2 ----
            uc2 = []
            mish2 = []
            for oc in range(2):
                u2a = nar.tile([128, BT], F32, tag="ua", name=f"ua2_{oc}")
                nc.vector.tensor_scalar(u2a[:], ps1[oc][:], USC,
                                        ubias1t[oc][:], ALU.mult, ALU.add)
                u2c = nar.tile([128, BT], F32, tag=f"uc2_{oc}", name=f"uc2_{oc}")
                nc.vector.tensor_scalar(u2c[:], u2a[:], 16.0, None, ALU.min)
                uc2.append(u2c)
                mish2.append(mish_of(ps1[oc][:], bias1t[oc][:], 128, f"L2_{oc}"))
                if dbg and bt == 0:
                    nc.sync.dma_start(dbg[f"h2_{oc}"][:], mish_of.last_h[:])

            cu2 = [wide_powers(uc2[ic], 128, NS, list(range(NS)), None,
                               f"L2_{ic}", cube_on_pool=(ic == 1))
                   for ic in range(2)]
            if dbg and bt == 0:
                nc.sync.dma_start(dbg["cu2_0"][:], cu2[0][:])
                nc.sync.dma_start(dbg["mish2_0"][:], mish2[0][:])

            ps2 = [ps.tile([128, BT], F32, tag=f"ps2_{oc}", name=f"ps2_{oc}") for oc in range(2)]
            for oc in range(2):
                first = True
                for ic in range(2):
                    for s in range(NS):
                        nc.tensor.matmul(
                            ps2[oc][:],
                            e2t[ic][:, s * 256 + oc * 128: s * 256 + (oc + 1) * 128],
                            cu2[ic][:, s * BT:(s + 1) * BT],
                            start=first, stop=False)
                        first = False
                for ic in range(2):
                    nc.tensor.matmul(ps2[oc][:],
                                     sb2t[ic][:, oc * 128:(oc + 1) * 128],
                                     mish2[ic][:], start=False, stop=(ic == 1))

            # ---- layer 3 ----
            uc3 = []
            mish3 = []
            for ic in range(2):
                u3a = nar.tile([128, BT], F32, tag="ua", name=f"ua3_{ic}")
                nc.vector.tensor_scalar(u3a[:], ps2[ic][:], USC,
                                        ubias2t[ic][:], ALU.mult, ALU.add)
                u3c = nar.tile([128, BT], F32, tag=f"uc3_{ic}", name=f"uc3_{ic}")
                nc.vector.tensor_scalar(u3c[:], u3a[:], 16.0, None, ALU.min)
                uc3.append(u3c)
                mish3.append(mish_of(ps2[ic][:], bias2t[ic][:], 128, f"L3_{ic}"))
                if dbg and bt == 0 and ic == 0:
                    nc.sync.dma_start(dbg["h3_0"][:], mish_of.last_h[:])

            cu3 = [wide_powers(uc3[ic], 128, NS, list(range(NS)), None,
                               f"L3_{ic}", cube_on_pool=(ic == 1))
                   for ic in range(2)]

            ps3 = ps.tile([10, BT], F32, tag="ps3", name="ps3")
            first = True
            for ic in range(2):
                for s in range(NS):
                    nc.tensor.matmul(ps3[:], e3t[ic][:, s * 10:(s + 1) * 10],
                                     cu3[ic][:, s * BT:(s + 1) * BT],
                                     start=first, stop=False)
                    first = False
            for ic in range(2):
                nc.tensor.matmul(ps3[:], sb3t[ic][:], mish3[ic][:],
                                 start=False, stop=(ic == 1))

            # logits (10, BT) + bias -> sbuf
            lg = sm.tile([10, BT], F32, tag="lg", name="lg")
            nc.vector.tensor_scalar(lg[:], ps3[:], bias3t[:], None, ALU.add)

            # ---- log_softmax + output ----
            for c4 in range(BT // 128):
                tp = ps.tile([128, 10], F32, tag="tp", name="tp")
                nc.tensor.transpose(tp[:], lg[:, c4 * 128:(c4 + 1) * 128],
                                    eyet[0:10, 0:10])
                t = sm.tile([128, 10], F32, tag="t", name="t")
                nc.scalar.activation(t[:], tp[:], AF.Copy)
                mx = sm.tile([128, 1], F32, tag="mx", name="mx")
                nc.vector.reduce_max(mx[:], t[:], axis=mybir.AxisListType.X)
                nmx = sm.tile([128, 1], F32, tag="nmx", name="nmx")
                nc.vector.tensor_scalar(nmx[:], mx[:], -1.0, None, ALU.mult)
                ex = sm.tile([128, 10], F32, tag="ex", name="ex")
                nc.scalar.activation(ex[:], t[:], AF.Exp, bias=nmx[:])
                ssum = sm.tile([128, 1], F32, tag="ssum", name="ssum")
                nc.vector.reduce_sum(ssum[:], ex[:], axis=mybir.AxisListType.X)
                lns = sm.tile([128, 1], F32, tag="lns", name="lns")
                nc.scalar.activation(lns[:], ssum[:], AF.Ln)
                off = sm.tile([128, 1], F32, tag="off", name="off")
                nc.vector.tensor_sub(off[:], nmx[:], lns[:])
                res = sm.tile([128, 10], F32, tag="res", name="res")
                nc.vector.tensor_scalar(res[:], t[:], off[:], None, ALU.add)
                nc.sync.dma_start(
                    out_d.ap()[bt * BT + c4 * 128: bt * BT + (c4 + 1) * 128, :],
                    res[:])

    nc.finalize()
    return nc


def kernel(**inputs):
    x = np.asarray(inputs['x'], np.float32)
    B = x.shape[0]
    pooled = x.reshape(B, 7, 4, 7, 4).mean(axis=(2, 4)).reshape(B, 49)
    xT = np.ascontiguousarray(pooled.T)                   # (49, 8192)

    key = 'nc'
    if key not in _CACHE:
        _CACHE[key] = _build(inputs)
    nc = _CACHE[key]

    in_maps = [{"xT": np.ascontiguousarray(
        xT[:, c * B_CORE:(c + 1) * B_CORE])} for c in range(N_CORES)]
    res = run_bass_kernel_spmd(nc, in_maps, core_ids=list(range(N_CORES)))
    out = np.concatenate([res.results[c]["out"] for c in range(N_CORES)], axis=0)
    return out.astype(np.float32)


if __name__ == "__main__":
    d = np.load('/root/problem/ref_data.npz')
    inputs = {k: d[k] for k in d.files if k != 'expected'}
    out = kernel(**inputs)
    exp = d['expected']
    err = np.abs(out - exp).max()
    rel = err / np.abs(exp).max()
    print(f"maxabs={err:.6g} rel={rel:.3g}")

